# revision 3
# baseline (speedup 1.0000x reference)
"""Trainium2 Bass kernel for a 4-layer GraphConv GNN (DGL norm='both') + max-pool + FC + softmax.

Strategy (8 NeuronCores, SPMD):
  - Nodes are partitioned across the 8 cores (graph/data parallel). The host sorts
    nodes by in-degree and deals them round-robin to cores, so per-core edge counts
    are balanced and nodes within a 128-node chunk have near-identical in-degree
    (minimizes gather padding).
  - Edges are sharded by dst: each core aggregates for its own 12.5k dst nodes.
  - Per layer, each core computes z = (h * ninv_out) @ W for its own nodes
    (PE transpose + matmul), then an AllGather replicates the z-table (bf16) into
    DRAM on every core. The segment-sum is computed by an indirect-DMA gather of
    z[src] rows (padded to a per-chunk-uniform width, pad slots point at a zero
    row) followed by a strided vector-engine reduce_sum. Epilogue:
    h = relu(m * ninv_in + b).
  - Degrees are computed on-device by reducing 0/1 edge-existence masks.
  - Final graph max-pool: per-chunk PE transpose + reduce_max, AllReduce(max)
    across cores, then the tiny FC + softmax computed on every core.
"""

import os
import sys

sys.path.insert(0, "/opt/trn_rl_repo")

import numpy as np
import ml_dtypes

import concourse.bass as bass
import concourse.bacc as bacc
import concourse.tile as tile
from concourse import mybir
from concourse.masks import make_identity
from concourse import bass_utils

F32 = mybir.dt.float32
BF16 = mybir.dt.bfloat16
I32 = mybir.dt.int32

N_NODES = 100000
N_EDGES = 3200000
IN_DIM = 128
HID = 32
N_CLASSES = 8
N_CORES = 8
P = 128

# dtype of the replicated z-table that gathers read (bf16 halves gather traffic;
# all accumulation stays fp32)
TABLE_DT = BF16
TABLE_NP = ml_dtypes.bfloat16

BATCH_W = 288  # max gather-slot columns per indirect DMA

LAST_RESULTS = None  # BassKernelResults of the most recent run (for profiling)


# --------------------------------------------------------------------------
# Host-side layout planning (pure numpy; graph-structure preprocessing only)
# --------------------------------------------------------------------------
def make_plan(src, dst, n_nodes, n_cores):
    src = np.asarray(src).astype(np.int64)
    dst = np.asarray(dst).astype(np.int64)
    assert n_nodes % (n_cores) == 0
    real = n_nodes // n_cores                      # real nodes per core
    slot = ((real + P - 1) // P) * P               # padded slots per core
    J = slot // P                                  # chunks of 128 nodes per core

    deg_in = np.bincount(dst, minlength=n_nodes).astype(np.int64)
    deg_out = np.bincount(src, minlength=n_nodes).astype(np.int64)

    # global rank by descending in-degree; rank g -> core g%C, local rank r=g//C
    order = np.argsort(-deg_in, kind="stable")
    rank = np.empty(n_nodes, dtype=np.int64)
    rank[order] = np.arange(n_nodes)
    core_of = (rank % n_cores).astype(np.int64)
    r_local = rank // n_cores
    j_of = r_local // P
    p_of = r_local % P
    # table row of a node (z-table layout: per-core block, partition-major)
    trow = core_of * slot + p_of * J + j_of
    ZR = n_cores * slot                            # zero row index
    n_tab = ZR + 1

    deg_in_by_rank = deg_in[order]
    deg_out_by_rank = deg_out[order]
    G = P * n_cores
    # uniform (across cores) in-degree pad width per chunk
    DP = np.zeros(J, dtype=np.int64)
    DPo = np.zeros(J, dtype=np.int64)
    for j in range(J):
        lo, hi = j * G, min((j + 1) * G, n_nodes)
        DP[j] = max(1, int(deg_in_by_rank[lo:hi].max()) if hi > lo else 1)
        DPo[j] = max(1, int(deg_out_by_rank[lo:hi].max()) if hi > lo else 1)
    off = np.zeros(J + 1, dtype=np.int64)
    off[1:] = np.cumsum(DP)
    offo = np.zeros(J + 1, dtype=np.int64)
    offo[1:] = np.cumsum(DPo)
    SD = int(off[J])
    SDo = int(offo[J])

    # batches of consecutive chunks for gather instructions
    batches = []  # (j_start, j_end, col_start, width)
    jb = 0
    while jb < J:
        je = jb
        w = 0
        while je < J and (je == jb or w + DP[je] <= BATCH_W):
            w += DP[je]
            je += 1
        batches.append((jb, je, int(off[jb]), int(w)))
        jb = je

    # gather indices: for each core, [P, SD] int32, pad slots -> ZR
    dslot = core_of[dst] * slot + p_of[dst] * J + j_of[dst]  # per-edge dst slot id (global)
    esort = np.argsort(dslot, kind="stable")
    ds_sorted = dslot[esort]
    srow_sorted = trow[src[esort]].astype(np.int32)
    # position of each edge within its dst group
    grp_start_idx = np.flatnonzero(np.r_[True, ds_sorted[1:] != ds_sorted[:-1]])
    grp_id = np.cumsum(np.r_[True, ds_sorted[1:] != ds_sorted[:-1]]) - 1
    pos_in_grp = np.arange(len(esort)) - grp_start_idx[grp_id]

    gcore = ds_sorted // slot
    lslot = ds_sorted % slot
    gp = lslot // J
    gj = lslot % J
    col = off[gj] + pos_in_grp

    ind = np.full((n_cores, P, SD), ZR, dtype=np.int32)
    ind[gcore, gp, col] = srow_sorted

    # masks: 1.0 for real edge slots (in), unary out-degree encoding (out)
    mask_in = np.zeros((n_cores, P, SD), dtype=TABLE_NP)
    mask_in[gcore, gp, col] = 1.0

    mask_out = np.zeros((n_cores, P, SDo), dtype=TABLE_NP)
    # node at (c, p, j) has global rank g = (j*P+p)*C + c
    ranks_grid = (np.arange(J)[None, :] * P + np.arange(P)[:, None]) * n_cores  # [P, J]
    for c in range(n_cores):
        g = ranks_grid + c
        valid = g < n_nodes
        dout = np.where(valid, deg_out_by_rank[np.minimum(g, n_nodes - 1)], 0)  # [P, J]
        for j in range(J):
            w = int(DPo[j])
            ar = np.arange(w)[None, :]
            mask_out[c, :, offo[j]:offo[j] + w] = (ar < dout[:, j:j + 1]).astype(TABLE_NP)

    # node id at (c, p, j) for x permutation; -1 for pads
    node_at = np.full((n_cores, P, J), -1, dtype=np.int64)
    for c in range(n_cores):
        g = ranks_grid + c
        valid = g < n_nodes
        node_at[c][valid] = order[np.minimum(g, n_nodes - 1)][valid]

    return dict(
        n_cores=n_cores, real=real, slot=slot, J=J, SD=SD, SDo=SDo,
        DP=DP, DPo=DPo, off=off, offo=offo, batches=batches,
        ZR=ZR, n_tab=n_tab, ind=ind, mask_in=mask_in, mask_out=mask_out,
        node_at=node_at,
    )


# --------------------------------------------------------------------------
# Bass program
# --------------------------------------------------------------------------
def build_program(plan, in_dim, hid, n_classes):
    J = plan["J"]
    SD, SDo = plan["SD"], plan["SDo"]
    DP, DPo = plan["DP"], plan["DPo"]
    off, offo = plan["off"], plan["offo"]
    batches = plan["batches"]
    ZR, n_tab = plan["ZR"], plan["n_tab"]
    real, slot = plan["real"], plan["slot"]
    n_cores = plan["n_cores"]
    H = hid

    nc = bacc.Bacc("TRN2", target_bir_lowering=False, debug=False,
                   num_devices=n_cores)

    x_d = nc.dram_tensor("x_sh", [P, J * in_dim], F32, kind="ExternalInput")
    ind_d = nc.dram_tensor("ind", [P, SD], I32, kind="ExternalInput")
    min_d = nc.dram_tensor("mask_in", [P, SD], TABLE_DT, kind="ExternalInput")
    mout_d = nc.dram_tensor("mask_out", [P, SDo], TABLE_DT, kind="ExternalInput")
    W0_d = nc.dram_tensor("W0", [in_dim, H], F32, kind="ExternalInput")
    Wl_d = [nc.dram_tensor(f"W{l}", [H, H], F32, kind="ExternalInput") for l in (1, 2, 3)]
    Wfc_d = nc.dram_tensor("Wfc", [H, n_classes], F32, kind="ExternalInput")
    b_d = [nc.dram_tensor(f"b{l}", [1, H], F32, kind="ExternalInput") for l in range(4)]
    bfc_d = nc.dram_tensor("bfc", [1, n_classes], F32, kind="ExternalInput")
    out_d = nc.dram_tensor("out", [1, n_classes], F32, kind="ExternalOutput")

    zshard = nc.dram_tensor("zshard", [slot, H], TABLE_DT, kind="Internal")
    table = nc.dram_tensor("ztable", [n_tab, H], TABLE_DT, kind="Internal",
                           addr_space="Shared")
    gmax_l = nc.dram_tensor("gmax_l", [HIDP(H), 1], F32, kind="Internal")
    gmax_g = nc.dram_tensor("gmax_g", [HIDP(H), 1], F32, kind="Internal",
                            addr_space="Shared")

    groups = [list(range(n_cores))]

    with tile.TileContext(nc) as tc:
        import contextlib
        with contextlib.ExitStack() as ctx:
            cpool = ctx.enter_context(tc.tile_pool(name="const", bufs=1))
            bigp = ctx.enter_context(tc.tile_pool(name="big", bufs=1))
            gatp = ctx.enter_context(tc.tile_pool(name="gat", bufs=2))
            xp = ctx.enter_context(tc.tile_pool(name="xs", bufs=3))
            scr = ctx.enter_context(tc.tile_pool(name="scr", bufs=3))
            psp = ctx.enter_context(tc.tile_pool(name="ps", bufs=2, space="PSUM"))

            # ---- constants ----
            ident = cpool.tile([P, P], F32)
            make_identity(nc, ident[:])
            ones_row = cpool.tile([1, P], F32)
            nc.gpsimd.memset(ones_row[:], 1.0)
            zrow = cpool.tile([1, H], TABLE_DT)
            nc.gpsimd.memset(zrow[:], 0.0)
            nc.sync.dma_start(out=table.ap()[ZR:ZR + 1, :], in_=zrow[:])

            W0_t = cpool.tile([in_dim, H], F32)
            nc.sync.dma_start(out=W0_t[:], in_=W0_d.ap()[:, :])
            Wl_t = []
            for l in range(3):
                w = cpool.tile([H, H], F32, tag=f"W{l + 1}")
                nc.sync.dma_start(out=w[:], in_=Wl_d[l].ap()[:, :])
                Wl_t.append(w)
            Wfc_t = cpool.tile([H, n_classes], F32)
            nc.sync.dma_start(out=Wfc_t[:], in_=Wfc_d.ap()[:, :])
            bfc_t = cpool.tile([1, n_classes], F32)
            nc.sync.dma_start(out=bfc_t[:], in_=bfc_d.ap()[:, :])

            # bias tiles broadcast to [P, H] via ones-matmul
            b_tiles = []
            for l in range(4):
                brow = cpool.tile([1, H], F32, tag=f"brow{l}")
                nc.sync.dma_start(out=brow[:], in_=b_d[l].ap()[:, :])
                bps = psp.tile([P, H], F32, tag="zp")
                nc.tensor.matmul(out=bps[:], lhsT=ones_row[:], rhs=brow[:],
                                 start=True, stop=True)
                bt = cpool.tile([P, H], F32, tag=f"btile{l}")
                nc.vector.tensor_copy(out=bt[:], in_=bps[:])
                b_tiles.append(bt)

            # ---- gather indices (resident, reused all layers) ----
            ind_t = bigp.tile([P, SD], I32)
            nc.sync.dma_start(out=ind_t[:], in_=ind_d.ap()[:, :])

            # ---- degrees from masks; ninv = sqrt(1/clip(deg,1)) ----
            def make_ninv(mask_dram, SDx, offx, DPx, tagp):
                mt = bigp.tile([P, SDx], TABLE_DT, tag=f"mask{tagp}")
                nc.sync.dma_start(out=mt[:], in_=mask_dram.ap()[:, :])
                deg = bigp.tile([P, J], F32, tag=f"deg{tagp}")
                for j in range(J):
                    nc.vector.reduce_sum(
                        out=deg[:, j:j + 1],
                        in_=mt[:, int(offx[j]):int(offx[j] + DPx[j])],
                        axis=mybir.AxisListType.X)
                nc.vector.tensor_scalar_max(out=deg[:], in0=deg[:], scalar1=1.0)
                rec = bigp.tile([P, J], F32, tag=f"rec{tagp}")
                nc.vector.reciprocal(out=rec[:], in_=deg[:])
                ninv = bigp.tile([P, J], F32, tag=f"ninv{tagp}")
                nc.scalar.activation(out=ninv[:], in_=rec[:],
                                     func=mybir.ActivationFunctionType.Sqrt)
                return ninv

            ninv_in = make_ninv(min_d, SD, off, DP, "i")
            ninv_out = make_ninv(mout_d, SDo, offo, DPo, "o")

            h_sb = bigp.tile([P, J * H], F32)
            z_sb = bigp.tile([P, J * H], TABLE_DT)

            # ---- layer 0 local: z0 = (x * ninv_out) @ W0 ----
            for j in range(J):
                xt_in = xp.tile([P, in_dim], F32, tag="xin")
                nc.sync.dma_start(out=xt_in[:],
                                  in_=x_d.ap()[:, j * in_dim:(j + 1) * in_dim])
                xs = xp.tile([P, in_dim], F32, tag="xsc")
                nc.vector.tensor_scalar_mul(out=xs[:], in0=xt_in[:],
                                            scalar1=ninv_out[:, j:j + 1])
                tp = psp.tile([P, P], F32, tag="tp")
                nc.tensor.transpose(out=tp[:], in_=xs[:], identity=ident[:])
                xt = scr.tile([P, in_dim], F32, tag="xT")
                nc.vector.tensor_copy(out=xt[:], in_=tp[:])
                zp = psp.tile([P, H], F32, tag="zp")
                nc.tensor.matmul(out=zp[:], lhsT=xt[:], rhs=W0_t[:],
                                 start=True, stop=True)
                nc.scalar.activation(out=z_sb[:, j * H:(j + 1) * H], in_=zp[:],
                                     func=mybir.ActivationFunctionType.Copy)

            zshard_ap = zshard.ap().rearrange("(p j) f -> p (j f)", p=P)

            for layer in range(4):
                # publish z -> all-gather into the replicated table
                nc.sync.dma_start(out=zshard_ap, in_=z_sb[:])
                nc.gpsimd.collective_compute(
                    "AllGather", mybir.AluOpType.bypass,
                    replica_groups=groups,
                    ins=[zshard.ap()[:, :]],
                    outs=[table.ap()[0:ZR, :]],
                )

                # aggregation: gather z[src] and segment-sum by dst chunk
                for (jb, je, c0, w) in batches:
                    gat = gatp.tile([P, BATCH_W * H], TABLE_DT, tag="gbuf")
                    # HW indirect DMA consumes ONE index per partition per
                    # instruction (walrus unrolls the dest outer dim), so
                    # gather each slot-column separately (128 rows/instr).
                    for col in range(w):
                        nc.gpsimd.indirect_dma_start(
                            out=gat[:, col * H:(col + 1) * H],
                            out_offset=None,
                            in_=table.ap()[:, :],
                            in_offset=bass.IndirectOffsetOnAxis(
                                ap=ind_t[:, c0 + col:c0 + col + 1], axis=0),
                        )
                    s0 = 0
                    for j in range(jb, je):
                        dj = int(DP[j])
                        m = scr.tile([P, H], F32, tag="m")
                        gv = gat[:, s0 * H:(s0 + dj) * H].rearrange(
                            "p (d f) -> p f d", d=dj, f=H)
                        nc.vector.reduce_sum(out=m[:], in_=gv,
                                             axis=mybir.AxisListType.X)
                        hpre = scr.tile([P, H], F32, tag="hpre")
                        nc.vector.scalar_tensor_tensor(
                            out=hpre[:], in0=m[:],
                            scalar=ninv_in[:, j:j + 1],
                            in1=b_tiles[layer][:],
                            op0=mybir.AluOpType.mult,
                            op1=mybir.AluOpType.add)
                        nc.scalar.activation(
                            out=h_sb[:, j * H:(j + 1) * H], in_=hpre[:],
                            func=mybir.ActivationFunctionType.Relu)
                        s0 += dj

                if layer < 3:
                    # z = (h * ninv_out) @ W_{layer+1}
                    for j in range(J):
                        hs = scr.tile([P, H], F32, tag="hs")
                        nc.vector.tensor_scalar_mul(
                            out=hs[:], in0=h_sb[:, j * H:(j + 1) * H],
                            scalar1=ninv_out[:, j:j + 1])
                        tp = psp.tile([P, P], F32, tag="tp")
                        nc.tensor.transpose(out=tp[:H, :], in_=hs[:],
                                            identity=ident[:])
                        hts = scr.tile([H, P], F32, tag="hts")
                        nc.vector.tensor_copy(out=hts[:], in_=tp[:H, :])
                        zp = psp.tile([P, H], F32, tag="zp")
                        nc.tensor.matmul(out=zp[:], lhsT=hts[:],
                                         rhs=Wl_t[layer][:],
                                         start=True, stop=True)
                        nc.scalar.activation(
                            out=z_sb[:, j * H:(j + 1) * H], in_=zp[:],
                            func=mybir.ActivationFunctionType.Copy)

            # ---- graph max-pool over real nodes ----
            pm = bigp.tile([HIDP(H), J], F32)
            for j in range(J):
                tp = psp.tile([P, P], F32, tag="tp")
                nc.tensor.transpose(out=tp[:H, :], in_=h_sb[:, j * H:(j + 1) * H],
                                    identity=ident[:])
                hts = scr.tile([H, P], F32, tag="hts")
                nc.vector.tensor_copy(out=hts[:], in_=tp[:H, :])
                ncols = P if (j + 1) * P <= real else max(0, real - j * P)
                if ncols > 0:
                    nc.vector.reduce_max(out=pm[:H, j:j + 1], in_=hts[:, :ncols],
                                         axis=mybir.AxisListType.X)
                else:
                    nc.vector.memset(pm[:H, j:j + 1], -1e30)
            gmax = scr.tile([HIDP(H), 1], F32, tag="gmax")
            nc.vector.reduce_max(out=gmax[:H, :], in_=pm[:H, :],
                                 axis=mybir.AxisListType.X)
            if HIDP(H) > H:
                nc.vector.memset(gmax[H:, :], 0.0)
            nc.sync.dma_start(out=gmax_l.ap()[:, :], in_=gmax[:])
            nc.gpsimd.collective_compute(
                "AllReduce", mybir.AluOpType.max,
                replica_groups=groups,
                ins=[gmax_l.ap()[:, :]],
                outs=[gmax_g.ap()[:, :]],
            )
            g_sb = scr.tile([HIDP(H), 1], F32, tag="gsb")
            nc.sync.dma_start(out=g_sb[:], in_=gmax_g.ap()[:, :])

            # ---- logits + softmax (replicated on every core) ----
            lgp = psp.tile([1, n_classes], F32, tag="lg")
            nc.tensor.matmul(out=lgp[:], lhsT=g_sb[:H, :], rhs=Wfc_t[:],
                             start=True, stop=True)
            lg = scr.tile([1, n_classes], F32, tag="lg1")
            nc.vector.tensor_copy(out=lg[:], in_=lgp[:])
            lgb = scr.tile([1, n_classes], F32, tag="lg2")
            nc.vector.tensor_add(out=lgb[:], in0=lg[:], in1=bfc_t[:])
            mx = scr.tile([1, 1], F32, tag="mx")
            nc.vector.reduce_max(out=mx[:], in_=lgb[:], axis=mybir.AxisListType.X)
            sh = scr.tile([1, n_classes], F32, tag="sh")
            nc.vector.tensor_scalar(out=sh[:], in0=lgb[:], scalar1=mx[:],
                                    scalar2=None, op0=mybir.AluOpType.subtract)
            ex = scr.tile([1, n_classes], F32, tag="ex")
            nc.scalar.activation(out=ex[:], in_=sh[:],
                                 func=mybir.ActivationFunctionType.Exp)
            sm = scr.tile([1, 1], F32, tag="sm")
            nc.vector.reduce_sum(out=sm[:], in_=ex[:], axis=mybir.AxisListType.X)
            rs = scr.tile([1, 1], F32, tag="rs")
            nc.vector.reciprocal(out=rs[:], in_=sm[:])
            so = scr.tile([1, n_classes], F32, tag="so")
            nc.vector.tensor_scalar_mul(out=so[:], in0=ex[:], scalar1=rs[:])
            nc.sync.dma_start(out=out_d.ap()[:, :], in_=so[:])

    nc.compile()
    return nc


def HIDP(h):
    # partition-dim padding for tiny per-feature vectors (keep 32 as-is)
    return h


# --------------------------------------------------------------------------
# Host wrapper
# --------------------------------------------------------------------------
def _make_in_maps(plan, x, W0, b0, W1, b1, W2, b2, W3, b3, Wfc, bfc, in_dim, hid,
                  n_classes):
    n_cores, J, slotn = plan["n_cores"], plan["J"], plan["slot"]
    node_at = plan["node_at"]
    in_maps = []
    for c in range(n_cores):
        na = node_at[c]  # [P, J]
        xs = np.zeros((P, J, in_dim), dtype=np.float32)
        valid = na >= 0
        xs[valid] = x[na[valid]]
        im = {
            "x_sh": xs.reshape(P, J * in_dim),
            "ind": plan["ind"][c],
            "mask_in": plan["mask_in"][c],
            "mask_out": plan["mask_out"][c],
            "W0": np.asarray(W0, dtype=np.float32),
            "W1": np.asarray(W1, dtype=np.float32),
            "W2": np.asarray(W2, dtype=np.float32),
            "W3": np.asarray(W3, dtype=np.float32),
            "Wfc": np.asarray(Wfc, dtype=np.float32),
            "b0": np.asarray(b0, dtype=np.float32).reshape(1, hid),
            "b1": np.asarray(b1, dtype=np.float32).reshape(1, hid),
            "b2": np.asarray(b2, dtype=np.float32).reshape(1, hid),
            "b3": np.asarray(b3, dtype=np.float32).reshape(1, hid),
            "bfc": np.asarray(bfc, dtype=np.float32).reshape(1, n_classes),
        }
        in_maps.append(im)
    return in_maps


LAST_NC = None
LAST_IN_MAPS = None


def kernel(x, src, dst, W0, b0, W1, b1, W2, b2, W3, b3, Wfc, bfc):
    global LAST_RESULTS, LAST_NC, LAST_IN_MAPS
    x = np.asarray(x, dtype=np.float32)
    assert x.shape == (N_NODES, IN_DIM)
    plan = make_plan(src, dst, N_NODES, N_CORES)
    nc = build_program(plan, IN_DIM, HID, N_CLASSES)
    in_maps = _make_in_maps(plan, x, W0, b0, W1, b1, W2, b2, W3, b3, Wfc, bfc,
                            IN_DIM, HID, N_CLASSES)
    LAST_NC, LAST_IN_MAPS = nc, in_maps
    res = bass_utils.run_bass_kernel_spmd(
        nc, in_maps, core_ids=list(range(N_CORES)),
        trace=bool(os.environ.get("GNN_TRACE")),
    )
    LAST_RESULTS = res
    return np.asarray(res.results[0]["out"], dtype=np.float32)
